# revision 1
# baseline (speedup 1.0000x reference)
"""Trainium2 Bass kernel for nn_AFE_78958678770209 (dense_cnn, deformable block).

Pipeline (per sample):
  h   = W1 @ x + b1           (W1 = def_w @ pw1_w @ dw1_w folded on host; the
                               three 1x1 convs commute with the bilinear gather)
  off = conv3x3(x, poff)      (offsets; bias folded into the base grid)
  g   = bilinear_gather(h, off)
  d2  = conv2x2_s2(g, dw2)    (dw2 bias folded into pw2 bias)
  out = pw2 @ d2 + b_out

Sharding: data-parallel over batch, 32 samples -> 8 cores x 4 samples.

Device layout notes:
  - All positions use the zero-padded 58x58 raster (3364 -> padded to
    3456 = 27*128 "position blocks").  Position s <-> (p=s%128, g=s//128).
  - h is produced TRANSPOSED (h^T[s, c]) directly by the matmul (lhsT = x-tile
    slice, rhs = W1^T), evicted bf16 into 512-byte rank stripes for the
    SBUF-source transpose-mode dma_gather (tokens_per_rank=128: token=p,
    rank=g).
  - offsets: A^T[s, (t,o)] = sum_c x[c,s] * poff[o,c,t] shares the lhsT with
    the h matmul; the 9-tap shifted sum goes through an HBM round trip (flat
    HBM addressing makes the +/-(dy*58+dx) position shift a strided DMA read).
  - bilinear combine in lerp form on DVE scalar_tensor_tensor ops, with wx/wy
    weight rows broadcast to 128 partitions via a K=2 selector matmul (PE) +
    ACT eviction.
"""

import numpy as np
import ml_dtypes

B, C, CO, H, W = 32, 192, 384, 56, 56
PH, PW = H + 2, W + 2              # 58x58 padded raster
NPAD = PH * PW                     # 3364
NPOS = 3456                        # padded to 27*128
NPOS2 = 3584                       # gather stream padded to 4*896
NG = NPOS // 128                   # 27 position blocks
NS = 4                             # samples per core
NCORES = 8
CPAD = 256                         # channel stripe (512B bf16) for gather elems
HOUT, WOUT = 28, 28
NOUT = HOUT * WOUT                 # 784
UT = 392                           # u-tile: 14 output rows x 28
APAD = 59                          # A_hbm row padding on each side

BF16 = ml_dtypes.bfloat16


def _fold_params(p):
    f32 = np.float32
    W1 = (p['def_w'].astype(f32) @ p['pw1_w'].astype(f32) @ p['dw1_w'].astype(f32))
    b1 = (p['def_w'].astype(f32) @ (p['pw1_w'].astype(f32) @ p['dw1_b'].astype(f32)
                                    + p['pw1_b'].astype(f32)) + p['def_b'].astype(f32))
    b_out = p['pw2_w'].astype(f32) @ p['dw2_b'].astype(f32) + p['pw2_b'].astype(f32)
    return W1, b1, b_out


def _sv(ap2d, boff, h, hstride, w, wstride):
    """Strided [P, h, w] view of a 2-dim AP [P, N] at element offset boff."""
    from bass_rust import AP
    return AP(ap2d.tensor, ap2d.offset + boff,
              [list(ap2d.ap[0]), [hstride, h], [wstride, w]])


def build_nc():
    import os
    import concourse.bacc as bacc
    import concourse.mybir as mybir
    import concourse.tile as tile

    STAGE = int(os.environ.get('KSTAGE', '9'))

    nc = bacc.Bacc("TRN2", target_bir_lowering=False, debug=False,
                   num_swdge_queues=1)
    dt = mybir.dt
    Alu = mybir.AluOpType
    f32, bf16, i16 = dt.float32, dt.bfloat16, dt.int16

    # ---------------- DRAM parameters ----------------
    x_d = nc.declare_dram_parameter("x", [NS, C + 1, NPOS], dt.bfloat16, isOutput=False)
    w1t_lo_d = nc.declare_dram_parameter("w1t_lo", [128, C], dt.bfloat16, isOutput=False)
    w1t_hi_d = nc.declare_dram_parameter("w1t_hi", [65, C], dt.bfloat16, isOutput=False)   # row64=b1
    wst_lo_d = nc.declare_dram_parameter("wst_lo", [128, 18], dt.bfloat16, isOutput=False)
    wst_hi_d = nc.declare_dram_parameter("wst_hi", [65, 18], dt.bfloat16, isOutput=False)  # row64=0
    k2t_d = nc.declare_dram_parameter("k2t", [4, C, C], dt.bfloat16, isOutput=False)       # [t,(c),(o)]
    pw2t_d = nc.declare_dram_parameter("pw2t", [C, CO], dt.bfloat16, isOutput=False)
    bout_d = nc.declare_dram_parameter("bout", [3, 128], dt.float32, isOutput=False)
    base_d = nc.declare_dram_parameter("base", [128, 2, NG], dt.float32, isOutput=False)
    sel_d = nc.declare_dram_parameter("sel", [2, 2, 128], dt.bfloat16, isOutput=False)
    out_d = nc.declare_dram_parameter("out", [NS, CO, NOUT], dt.bfloat16,
                                      isOutput=True)
    DBG = int(os.environ.get('KDBG', '0'))
    if DBG:
        dbg_off = nc.declare_dram_parameter("dbg_off", [128, 2, NG], dt.float32, isOutput=True)
        dbg_idx = nc.declare_dram_parameter("dbg_idx", [128, 4, NG + 1], dt.int16, isOutput=True)
        dbg_w = nc.declare_dram_parameter("dbg_w", [128, NPOS2], dt.bfloat16, isOutput=True)
        dbg_g = nc.declare_dram_parameter("dbg_g", [128, 2, NPOS2], dt.bfloat16, isOutput=True)
        dbg_G0 = nc.declare_dram_parameter("dbg_G0", [128, 4, 2, 896], dt.bfloat16, isOutput=True)
        dbg_hT = nc.declare_dram_parameter("dbg_hT", [128, NG, CPAD], dt.bfloat16, isOutput=True)

    TAP_SHIFT = [dy * PW + dx for dy in (-1, 0, 1) for dx in (-1, 0, 1)]

    with tile.TileContext(nc) as tc:
        with (
            tc.tile_pool(name="const", bufs=1) as cp,
            tc.tile_pool(name="x", bufs=1) as xp,
            tc.tile_pool(name="hT", bufs=2) as hp,
            tc.tile_pool(name="G", bufs=5) as gp,
            tc.tile_pool(name="sc", bufs=3) as sp,
            tc.tile_pool(name="W", bufs=2) as wp,
            tc.tile_pool(name="gsb", bufs=1) as gsp,
            tc.tile_pool(name="d2", bufs=1) as d2p,
            tc.tile_pool(name="osb", bufs=1) as op,
            tc.tile_pool(name="small", bufs=2) as ap_,
            tc.tile_pool(name="ps", bufs=6, space="PSUM") as ps,
            tc.tile_pool(name="psA", bufs=2, space="PSUM") as psA,
            tc.tile_pool(name="dram", bufs=2, space="DRAM") as dp,
        ):
            # ---------------- constants to SBUF ----------------
            w1t_lo = cp.tile([128, C], bf16)
            nc.sync.dma_start(w1t_lo[:], w1t_lo_d[:])
            w1t_hi = cp.tile([65, C], bf16)
            nc.sync.dma_start(w1t_hi[:], w1t_hi_d[:])
            wst_lo = cp.tile([128, 18], bf16)
            nc.sync.dma_start(wst_lo[:], wst_lo_d[:])
            wst_hi = cp.tile([65, 18], bf16)
            nc.sync.dma_start(wst_hi[:], wst_hi_d[:])
            k2t_lo = cp.tile([128, 4, C], bf16)
            nc.sync.dma_start(k2t_lo[:],
                              k2t_d[:, 0:128, :].rearrange("t c o -> c t o"))
            k2t_hi = cp.tile([64, 4, C], bf16)
            nc.sync.dma_start(k2t_hi[:],
                              k2t_d[:, 128:192, :].rearrange("t c o -> c t o"))
            pw2t_lo = cp.tile([128, CO], bf16)
            nc.sync.dma_start(pw2t_lo[:], pw2t_d[0:128, :])
            pw2t_hi = cp.tile([64, CO], bf16)
            nc.sync.dma_start(pw2t_hi[:], pw2t_d[128:192, :])
            bout = cp.tile([128, 3], f32)
            nc.sync.dma_start(bout[:], bout_d[:].rearrange("b p -> p b"))
            base = cp.tile([128, 2, NG], f32)
            nc.sync.dma_start(base[:], base_d[:])
            sel = cp.tile([2, 2, 128], bf16)
            nc.sync.dma_start(sel[:], sel_d[:])
            zsb = cp.tile([APAD, 18], f32)
            nc.vector.memset(zsb[:], 0.0)

            from concourse import library_config
            nc.gpsimd.load_library(library_config.mlp)

            for si in range(NS):
                qn = si % 4
                # ---------------- load x ----------------
                x_lo = xp.tile([128, NPOS], bf16, tag="xlo")
                x_hi = xp.tile([65, NPOS], bf16, tag="xhi")
                nc.sync.dma_start(x_lo[:], x_d[si, 0:128, :])
                nc.sync.dma_start(x_hi[:], x_d[si, 128:193, :])

                # ---------------- h^T and A^T matmuls ----------------
                hT = hp.tile([128, NG, CPAD], bf16, tag="hT")
                nc.vector.memset(hT[:, :, C:CPAD], 0.0)
                A_sb = ap_.tile([128, NG, 18], f32, tag="Asb")
                for g in range(NG):
                    sl = slice(g * 128, (g + 1) * 128)
                    ph = ps.tile([128, C], f32, tag="ps")
                    pa = psA.tile([128, 18], f32, tag="psA")
                    nc.tensor.matmul(ph[:], x_lo[:, sl], w1t_lo[:],
                                     start=True, stop=False)
                    nc.tensor.matmul(pa[:], x_lo[:, sl], wst_lo[:],
                                     start=True, stop=False)
                    nc.tensor.matmul(ph[:], x_hi[:, sl], w1t_hi[:],
                                     start=False, stop=True)
                    nc.tensor.matmul(pa[:], x_hi[:, sl], wst_hi[:],
                                     start=False, stop=True)
                    nc.scalar.copy(hT[:, g, 0:C], ph[:])
                    nc.vector.tensor_copy(A_sb[:, g, :], pa[:])

                # ---------------- offsets: HBM shifted tap-sum ----------------
                A_hbm = dp.tile([NPOS + 2 * APAD, 18], f32, tag="Ahbm")
                nc.sync.dma_start(A_hbm[0:APAD, :], zsb[:])
                nc.sync.dma_start(
                    A_hbm[NPOS + APAD:NPOS + 2 * APAD, :], zsb[:])
                nc.sync.dma_start(
                    A_hbm[APAD:APAD + NPOS, :].rearrange("(g p) o -> p g o",
                                                         p=128),
                    A_sb[:])
                A_sh = ap_.tile([128, 9, NG, 2], f32, tag="Ash")
                for t in range(9):
                    src = A_hbm[APAD + TAP_SHIFT[t]:
                                APAD + TAP_SHIFT[t] + NPOS, 2 * t:2 * t + 2]
                    nc.sync.dma_start(
                        A_sh[:, t, :, :],
                        src.rearrange("(g p) o -> p g o", p=128))

                # ---------------- index math ----------------
                off = ap_.tile([128, 2, NG], f32, tag="off")
                nc.vector.tensor_reduce(
                    off[:], A_sh[:].rearrange("p t g o -> p o g t"),
                    mybir.AxisListType.X, Alu.add)
                pyx = ap_.tile([128, 2, NG], f32, tag="pyx")
                nc.vector.tensor_tensor(pyx[:], off[:], base[:], Alu.add)
                nc.vector.tensor_scalar(pyx[:], pyx[:], 0.0, float(H - 1),
                                        Alu.max, Alu.min)
                y0i = ap_.tile([128, 2, NG], dt.int32, tag="y0i")
                nc.vector.tensor_copy(y0i[:], pyx[:])
                icast = ap_.tile([128, 2, NG], f32, tag="icast")
                nc.vector.tensor_copy(icast[:], y0i[:])
                # round-mode-invariant floor: subtract 1 where cast rounded up
                gtt = ap_.tile([128, 2, NG], f32, tag="gtt")
                nc.vector.tensor_tensor(gtt[:], icast[:], pyx[:], Alu.is_gt)
                ifl = ap_.tile([128, 2, NG], f32, tag="ifl")
                nc.vector.tensor_tensor(ifl[:], icast[:], gtt[:], Alu.subtract)
                frac = ap_.tile([128, 2, NG], f32, tag="frac")
                nc.vector.tensor_tensor(frac[:], pyx[:], ifl[:], Alu.subtract)
                w_bf = ap_.tile([128, 2, NG], bf16, tag="wbf")
                nc.vector.tensor_copy(w_bf[:], frac[:])
                Bt = ap_.tile([128, NG], f32, tag="Bt")
                nc.vector.tensor_scalar(Bt[:], ifl[:, 0, :], float(PW),
                                        float(PW + 1), Alu.mult, Alu.add)
                nc.vector.tensor_tensor(Bt[:], Bt[:], ifl[:, 1, :], Alu.add)
                idx4 = ap_.tile([128, 4, NG + 1], i16, tag="idx4")
                nc.vector.memset(idx4[:, :, NG:NG + 1], 0)
                for v, add in enumerate((0.0, 1.0, float(PW), float(PW + 1))):
                    nc.vector.tensor_scalar(idx4[:, v, 0:NG], Bt[:], add, None,
                                            Alu.add)

                # reorder indices into the 16-partition-wrapped layout:
                # gather slot i = v*NPOS + s;  value for slot i must sit at
                # [i%16, i//16].  s = g*128 + 16*a + b  ->  dst[b, v, 8g+a].
                idx16 = ap_.tile([16, 4, 224], i16, tag="idx16")
                for a in range(8):
                    nc.sync.dma_start(idx16[:, :, a:224:8],
                                      idx4[16 * a:16 * (a + 1), :, :])
                idx_hbm = dp.tile([16, 896], i16, tag="idxhbm")
                nc.sync.dma_start(idx_hbm[:], idx16[:])
                idxs = ap_.tile([128, 4, 224], i16, tag="idxs")
                from bass_rust import AP as _AP
                _rep = _AP(idx_hbm[:].tensor, idx_hbm[:].offset,
                           [[0, 8], [896, 16], [1, 896]])
                nc.sync.dma_start(idxs[:].rearrange("p v t -> p (v t)"), _rep)

                # wx / wy rows -> HBM -> 2-row tile -> PE broadcast
                w_hbm = dp.tile([2, NPOS], bf16, tag="whbm")
                nc.sync.dma_start(
                    w_hbm[:].rearrange("o (g p) -> p o g", p=128), w_bf[:])
                w_rows = ap_.tile([2, NPOS2], bf16, tag="wrows")
                nc.sync.dma_start(w_rows[:, 0:NPOS], w_hbm[:])
                nc.vector.memset(w_rows[:, NPOS:NPOS2], 0.0)
                WY = wp.tile([128, NPOS2], bf16, tag="W")
                WX = wp.tile([128, NPOS2], bf16, tag="W")
                for o, Wt in ((0, WY), (1, WX)):
                    for n0 in range(0, NPOS2, 512):
                        n1 = min(n0 + 512, NPOS2)
                        pwt = ps.tile([128, 512], f32, tag="ps")
                        nc.tensor.matmul(pwt[:, 0:n1 - n0], sel[:, o, :],
                                         w_rows[:, n0:n1],
                                         start=True, stop=True)
                        nc.scalar.copy(Wt[:, n0:n1], pwt[:, 0:n1 - n0])

                # ---------------- gather (SBUF-source, transpose mode) -------
                # SWDGE ring caps s2m descs at 128 -> chunk to <=896 idxs.
                Gv = []
                for _gi in range(4):
                    Gt = gp.tile([128, 4, 2, 896], bf16, tag="G")
                    Gv.append(Gt)
                gq = 0
                for v in range(4):
                    for c in range(4):
                        nc.gpsimd.dma_gather(
                            Gv[v][:, c, :, :],
                            hT[:].rearrange("p g c -> p (g c)"),
                            idxs[:, v, c * 56:(c + 1) * 56],
                            num_idxs=896, num_idxs_reg=896,
                            elem_size=CPAD, transpose=True,
                            queue_num=0,
                            sbuf_tokens_per_rank=128,
                            sbuf_free_dim_per_rank=CPAD * 2)
                        gq += 1

                # ---------------- bilinear lerp combine (DVE stt) ------------
                g_sb = gsp.tile([128, 2, NPOS2], bf16, tag="gsb")
                stt = nc.vector.scalar_tensor_tensor
                for j in range(2):
                    L0, R0 = Gv[0][:, :, j, :], Gv[1][:, :, j, :]
                    L1, R1 = Gv[2][:, :, j, :], Gv[3][:, :, j, :]
                    d = sp.tile([128, NPOS2], bf16, tag="sc")
                    m = sp.tile([128, NPOS2], bf16, tag="sc")
                    t1 = sp.tile([128, NPOS2], bf16, tag="sc")
                    t0 = g_sb[:, j, :]
                    stt(d[:], R0, 1.0, L0, Alu.mult, Alu.subtract)
                    stt(m[:], d[:], 1.0, WX[:], Alu.mult, Alu.mult)
                    stt(t0, L0, 1.0, m[:], Alu.mult, Alu.add)
                    stt(d[:], R1, 1.0, L1, Alu.mult, Alu.subtract)
                    stt(m[:], d[:], 1.0, WX[:], Alu.mult, Alu.mult)
                    stt(t1[:], L1, 1.0, m[:], Alu.mult, Alu.add)
                    stt(d[:], t1[:], 1.0, t0, Alu.mult, Alu.subtract)
                    stt(m[:], d[:], 1.0, WY[:], Alu.mult, Alu.mult)
                    stt(t0, t0, 1.0, m[:], Alu.mult, Alu.add)

                if DBG and si == 0:
                    nc.sync.dma_start(dbg_off[:], off[:])
                    nc.sync.dma_start(dbg_idx[:], idx4[:])
                    nc.sync.dma_start(dbg_w[:], WX[:])
                    nc.sync.dma_start(dbg_g[:], g_sb[:])
                    nc.sync.dma_start(dbg_G0[:], Gv[0][:])
                    nc.sync.dma_start(dbg_hT[:], hT[:])

                # ---------------- dw2 (2x2 stride-2) -------------------------
                d2_lo = d2p.tile([128, NOUT], bf16, tag="d2lo")
                d2_hi = d2p.tile([64, NOUT], bf16, tag="d2hi")
                glo = g_sb[:, 0, :]
                ghi = g_sb[0:64, 1, :]
                for obase, osz, dtile in ((0, 128, d2_lo), (128, 64, d2_hi)):
                    for ut in range(2):
                        pd = ps.tile([osz, UT], f32, tag="ps")
                        for t in range(4):
                            dy, dx = t // 2, t % 2
                            boff = PW * (1 + dy) + (1 + dx) + ut * 14 * 2 * PW
                            rhs_lo = _sv(glo, boff, 14, 2 * PW, 28, 2)
                            rhs_hi = _sv(ghi, boff, 14, 2 * PW, 28, 2)
                            nc.tensor.matmul(
                                pd[:], k2t_lo[:, t, obase:obase + osz], rhs_lo,
                                start=(t == 0), stop=False)
                            nc.tensor.matmul(
                                pd[:], k2t_hi[:, t, obase:obase + osz], rhs_hi,
                                start=False, stop=(t == 3))
                        nc.scalar.copy(dtile[:, ut * UT:(ut + 1) * UT], pd[:])

                # ---------------- pw2 ----------------------------------------
                out_sb = op.tile([128, 3, NOUT], bf16, tag="osb")
                for o3 in range(3):
                    osl = slice(o3 * 128, (o3 + 1) * 128)
                    for ut in range(2):
                        usl = slice(ut * UT, (ut + 1) * UT)
                        po = ps.tile([128, UT], f32, tag="ps")
                        nc.tensor.matmul(po[:], pw2t_lo[:, osl],
                                         d2_lo[:, usl], start=True, stop=False)
                        nc.tensor.matmul(po[:], pw2t_hi[:, osl],
                                         d2_hi[:, usl], start=False, stop=True)
                        nc.scalar.add(out_sb[:, o3, usl], po[:],
                                      bout[:, o3:o3 + 1])
                nc.sync.dma_start(
                    out_d[si, :, :].rearrange("(b p) n -> p b n", p=128),
                    out_sb[:])

    nc.compile()
    return nc


def _prep_inputs(p):
    x = p['x'].astype(np.float32)
    W1, b1, b_out = _fold_params(p)

    xpad = np.zeros((B, C, PH, PW), np.float32)
    xpad[:, :, 1:PH - 1, 1:PW - 1] = x
    xflat = np.zeros((B, C + 1, NPOS), np.float32)
    xflat[:, 0:C, 0:NPAD] = xpad.reshape(B, C, NPAD)
    xflat[:, C, :] = 1.0
    xflat = xflat.astype(BF16)

    w1t_lo = np.ascontiguousarray(W1.T[0:128, :]).astype(BF16)
    w1t_hi = np.zeros((65, C), np.float32)
    w1t_hi[0:64] = W1.T[128:192, :]
    w1t_hi[64] = b1
    w1t_hi = w1t_hi.astype(BF16)

    poff = p['poff_w'].astype(np.float32)          # [2, C, 3, 3]
    wst = np.zeros((C, 18), np.float32)            # col = t*2 + o
    for t in range(9):
        dy, dx = t // 3, t % 3
        for o in range(2):
            wst[:, t * 2 + o] = poff[o, :, dy, dx]
    wst_lo = np.ascontiguousarray(wst[0:128]).astype(BF16)
    wst_hi = np.zeros((65, 18), np.float32)
    wst_hi[0:64] = wst[128:192]
    wst_hi = wst_hi.astype(BF16)

    dw2 = p['dw2_w'].astype(np.float32)            # [O, C, 2, 2]
    k2t = np.zeros((4, C, C), np.float32)
    for t in range(4):
        dy, dx = t // 2, t % 2
        k2t[t] = dw2[:, :, dy, dx].T               # [c, o]
    k2t = k2t.astype(BF16)

    pw2t = np.ascontiguousarray(p['pw2_w'].astype(np.float32).T).astype(BF16)
    bout = b_out.reshape(3, 128).astype(np.float32)

    s = np.arange(NPOS, dtype=np.float32)
    ypad = np.floor_divide(np.minimum(s, NPAD - 1), PW)
    xpad_c = np.minimum(s, NPAD - 1) % PW
    base = np.zeros((128, 2, NG), np.float32)
    base[:, 0, :] = (ypad - 1.0 + float(p['poff_b'][0])).reshape(NG, 128).T
    base[:, 1, :] = (xpad_c - 1.0 + float(p['poff_b'][1])).reshape(NG, 128).T

    sel = np.zeros((2, 2, 128), np.float32)
    sel[0, 0, :] = 1.0
    sel[1, 1, :] = 1.0
    sel = sel.astype(BF16)

    shared = dict(w1t_lo=w1t_lo, w1t_hi=w1t_hi, wst_lo=wst_lo, wst_hi=wst_hi,
                  k2t=k2t, pw2t=pw2t, bout=bout, base=base, sel=sel)
    in_maps = []
    for ci in range(NCORES):
        m = dict(shared)
        m['x'] = np.ascontiguousarray(xflat[ci * NS:(ci + 1) * NS])
        in_maps.append(m)
    return in_maps


def kernel(**inputs):
    from concourse.bass_utils import run_bass_kernel_spmd

    p = {k: np.asarray(v) for k, v in inputs.items()}
    in_maps = _prep_inputs(p)
    nc = build_nc()
    res = run_bass_kernel_spmd(nc, in_maps, core_ids=list(range(NCORES)))
    outs = [res.results[ci]['out'] for ci in range(NCORES)]
    out = np.concatenate([np.asarray(o).astype(np.float32) for o in outs],
                         axis=0)
    return out.reshape(B, CO, HOUT, WOUT)



# revision 18
# speedup vs baseline: 2.1780x; 2.1780x over previous
"""Trainium2 Bass kernel for nn_AFE_78958678770209 (dense_cnn, deformable block).

Pipeline (per sample):
  h   = W1 @ x + b1           (W1 = def_w @ pw1_w @ dw1_w folded on host)
  off = conv3x3(x, poff)      (offsets; bias folded into the base grid)
  g   = bilinear_gather(h, off)
  d2  = conv2x2_s2(g, dw2)    (dw2 bias folded into pw2 bias)
  out = pw2 @ d2 + b_out

Sharding: data-parallel over batch, 32 samples -> 8 cores x 4 samples.

v2 layout notes (all index math on-chip, no HBM round trips):
  - positions s on the zero-padded 58x58 raster, padded 3364 -> 3456 = 27*128;
    s <-> (p = s%128, g = s//128).
  - tokens are PAIRS: token t (partition t%128, rank t//128, 1024B stripe)
    holds [h(t) pad256 | h(t+1) pad256] bf16.  Produced by TWO matmul streams
    (lhsT = x[:, s0:s0+128] and x[:, s0+1:s0+129]) into one [128,384] PSUM,
    one ACT eviction per block.
  - bilinear corners (s, s+1, s+58, s+59) = tokens {s, s+58}: 2 descriptors
    of 1024B per output position (vs 4x512B before) -> half the SWDGE
    emission, which was the bottleneck.
  - offsets: A^T[s,(t,o)] matmul as before; the 9-tap shifted sum is done
    with partition-shifted DVE adds (was: HBM round trip with 31k 8B reads).
  - gather idx tile built with partition-fold DVE copies ([i%16, i//16]
    wrap) + on-chip replication (was: 14k 2B SBUF descs + HBM trip).
  - wx/wy: PE transpose + 54x256B SBUF-SBUF DMA + selector-matmul broadcast
    (was: 7k 2B HBM scatter).
  - combine in tensor_tensor ops (2x 16-bit DVE mode eligible; stt is not).
"""

import os
import numpy as np
import ml_dtypes

B, C, CO, H, W = 32, 192, 384, 56, 56
PH, PW = H + 2, W + 2              # 58x58 padded raster
NPAD = PH * PW                     # 3364
NPOS = 3456                        # padded to 27*128
NPOS2 = 3584                       # gather stream padded to 4*896
NG = NPOS // 128                   # 27 position blocks
NS = 4                             # samples per core
NCORES = 8
XW = 3584                          # x tile width (>= NPOS+1)
TOKB = 1024                        # token stripe bytes (2 pos x 256 bf16)
HOUT, WOUT = 28, 28
NOUT = HOUT * WOUT                 # 784
UT = 392                           # u-tile: 14 output rows x 28

BF16 = ml_dtypes.bfloat16


def _fold_params(p):
    f32 = np.float32
    W1 = (p['def_w'].astype(f32) @ p['pw1_w'].astype(f32) @ p['dw1_w'].astype(f32))
    b1 = (p['def_w'].astype(f32) @ (p['pw1_w'].astype(f32) @ p['dw1_b'].astype(f32)
                                    + p['pw1_b'].astype(f32)) + p['def_b'].astype(f32))
    b_out = p['pw2_w'].astype(f32) @ p['dw2_b'].astype(f32) + p['pw2_b'].astype(f32)
    return W1, b1, b_out


def _sv(ap2d, boff, h, hstride, w, wstride):
    """Strided [P, h, w] view of a 2-dim AP [P, N] at element offset boff."""
    from bass_rust import AP
    return AP(ap2d.tensor, ap2d.offset + boff,
              [list(ap2d.ap[0]), [hstride, h], [wstride, w]])


def _rap(ap, dims):
    """Raw AP with explicit free dims [(stride, n), ...] at ap's offset."""
    from bass_rust import AP
    return AP(ap.tensor, ap.offset, [list(ap.ap[0])] + [list(d) for d in dims])


def build_nc():
    import concourse.bacc as bacc
    import concourse.mybir as mybir
    import concourse.tile as tile

    NQ = int(os.environ.get('KQUEUES', '2'))
    NIDX = int(os.environ.get('KNIDX', '896'))     # idxs per gather call
    NCH = NPOS2 // (16 * (NIDX // 16))             # chunk count
    assert NPOS2 % NIDX == 0

    nc = bacc.Bacc("TRN2", target_bir_lowering=False, debug=False,
                   num_swdge_queues=NQ)
    dt = mybir.dt
    Alu = mybir.AluOpType
    f32, bf16, i16 = dt.float32, dt.bfloat16, dt.int16

    # ---------------- DRAM parameters ----------------
    x_d = nc.declare_dram_parameter("x", [NS, C + 1, XW], dt.bfloat16, isOutput=False)
    w1st_lo_d = nc.declare_dram_parameter("w1st_lo", [128, C + 18], dt.bfloat16, isOutput=False)
    w1st_hi_d = nc.declare_dram_parameter("w1st_hi", [65, C + 18], dt.bfloat16, isOutput=False)  # row64=[b1|0]
    k2t_d = nc.declare_dram_parameter("k2t", [4, C, C], dt.bfloat16, isOutput=False)       # [t,(c),(o)]
    pw2t_d = nc.declare_dram_parameter("pw2t", [C, CO], dt.bfloat16, isOutput=False)
    bout_d = nc.declare_dram_parameter("bout", [3, 128], dt.float32, isOutput=False)
    base_d = nc.declare_dram_parameter("base", [128, NG, 2], dt.float32, isOutput=False)
    sel_d = nc.declare_dram_parameter("sel", [2, 2, 128], dt.bfloat16, isOutput=False)
    ident_d = nc.declare_dram_parameter("ident", [128, 128], dt.bfloat16, isOutput=False)
    shifts_d = nc.declare_dram_parameter("shifts", [17, 128, 128], dt.bfloat16,
                                         isOutput=False)
    pfold_d = nc.declare_dram_parameter("pfold", [8, 128, 128], dt.float32,
                                        isOutput=False)
    out_d = nc.declare_dram_parameter("out", [NS, CO, NOUT], dt.bfloat16, isOutput=True)

    TAPS = [(t, dy * PW + dx)
            for t, (dy, dx) in enumerate((dy, dx) for dy in (-1, 0, 1)
                                         for dx in (-1, 0, 1))]

    from contextlib import ExitStack
    with ExitStack() as _stk:
        tc = _stk.enter_context(tile.TileContext(nc))
        _p = lambda **kw: _stk.enter_context(tc.tile_pool(**kw))
        cp = _p(name="const", bufs=1)
        xp = _p(name="x", bufs=1)
        ap_ = _p(name="A", bufs=1)
        ip = _p(name="idx", bufs=2)
        wcp = _p(name="wc", bufs=1)
        wp = _p(name="W", bufs=2)
        gp = _p(name="G", bufs=2)
        tp_ = _p(name="tt", bufs=2)
        sp = _p(name="sc", bufs=2)
        gsp = _p(name="gsb", bufs=1)
        d2p = _p(name="d2", bufs=1)
        op = _p(name="osb", bufs=1)
        dp = _p(name="dram", bufs=1, space="DRAM")
        ps = _p(name="ps", bufs=2, space="PSUM")
        pst = _p(name="pst", bufs=1, space="PSUM")
        psA = _p(name="psA", bufs=1, space="PSUM")
        psH = _p(name="psH", bufs=3, space="PSUM")
        if True:
            # ---------------- constants to SBUF ----------------
            w1st_lo = cp.tile([128, C + 18], bf16)
            nc.sync.dma_start(w1st_lo[:], w1st_lo_d[:])
            w1st_hi = cp.tile([65, C + 18], bf16)
            nc.sync.dma_start(w1st_hi[:], w1st_hi_d[:])
            k2t_lo = cp.tile([128, 4, C], bf16)
            nc.sync.dma_start(k2t_lo[:],
                              k2t_d[:, 0:128, :].rearrange("t c o -> c t o"))
            k2t_hi = cp.tile([64, 4, C], bf16)
            nc.sync.dma_start(k2t_hi[:],
                              k2t_d[:, 128:192, :].rearrange("t c o -> c t o"))
            pw2t_lo = cp.tile([128, CO], bf16)
            nc.sync.dma_start(pw2t_lo[:], pw2t_d[0:128, :])
            pw2t_hi = cp.tile([64, CO], bf16)
            nc.sync.dma_start(pw2t_hi[:], pw2t_d[128:192, :])
            bout = cp.tile([128, 3], f32)
            nc.sync.dma_start(bout[:], bout_d[:].rearrange("b p -> p b"))
            base = cp.tile([128, NG, 2], f32)
            nc.sync.dma_start(base[:], base_d[:])
            sel = cp.tile([2, 2, 128], bf16)
            nc.sync.dma_start(sel[:], sel_d[:])
            ident = cp.tile([128, 128], bf16)
            nc.sync.dma_start(ident[:], ident_d[:])
            shifts = cp.tile([128, 17, 128], bf16)
            nc.sync.dma_start(shifts[:], shifts_d[:].rearrange("t k m -> k t m"))
            pfold = cp.tile([128, 8, 128], f32)
            nc.sync.dma_start(pfold[:], pfold_d[:].rearrange("t k m -> k t m"))

            from concourse import library_config
            nc.gpsimd.load_library(library_config.mlp)

            # token pair buffers (manual ping-pong; pads zeroed once)
            tok_bufs = []
            for _tb in range(2):
                _tok = cp.tile([128, NG, 512], bf16, tag=f"tokbuf{_tb}")
                nc.vector.memset(
                    _rap(_tok[:, 0, 192], [(256, 2 * NG), (1, 64)]), 0.0)
                tok_bufs.append(_tok)

            for si in range(NS):
                # ---------------- load x ----------------
                x_lo = xp.tile([128, XW], bf16, tag="xlo")
                x_hi = xp.tile([65, XW], bf16, tag="xhi")
                nc.sync.dma_start(x_lo[:], x_d[si, 0:128, :])
                nc.sync.dma_start(x_hi[:], x_d[si, 128:193, :])

                # ---------------- h pair-tokens + A^T matmuls ----------------
                # token t: [h(t) | pad | h(t+1) | pad], 512 bf16 = 1024B
                tok = tok_bufs[si % 2]
                A_sb = ap_.tile([128, NG + 1, 18], bf16, tag="Asb")
                nc.vector.memset(A_sb[:, NG, :], 0.0)

                for g in range(NG):
                    s0 = g * 128
                    ph = psH.tile([128, 2 * C + 18], f32, tag="psH")
                    nc.tensor.matmul(ph[:, 0:C + 18], x_lo[:, s0:s0 + 128],
                                     w1st_lo[:], start=True, stop=False)
                    nc.tensor.matmul(ph[:, 0:C + 18], x_hi[:, s0:s0 + 128],
                                     w1st_hi[:], start=False, stop=True)
                    nc.tensor.matmul(ph[:, C + 18:2 * C + 18],
                                     x_lo[:, s0 + 1:s0 + 129],
                                     w1st_lo[:, 0:C], start=True, stop=False)
                    nc.tensor.matmul(ph[:, C + 18:2 * C + 18],
                                     x_hi[:, s0 + 1:s0 + 129],
                                     w1st_hi[:, 0:C], start=False, stop=True)
                    # evict: tok[p, g, {0:192, 256:448}] = h(s), h(s+1)
                    nc.scalar.copy(
                        _rap(tok[:, g, 0], [(256, 2), (1, C)]),
                        _rap(ph[:, 0], [(C + 18, 2), (1, C)]))
                    nc.vector.tensor_copy(A_sb[:, g, :], ph[:, C:C + 18])

                # ------- 9-tap shifted sum via PE one-hot shift matmuls -------
                ps_off = psA.tile([128, 54], f32, tag="psOff")
                ofull = _rap(ps_off[:, 0], [(2, NG), (1, 2)])
                nc.tensor.matmul(ofull, shifts[:, 0, :],
                                 A_sb[:, 0:NG, 8:10], start=True, stop=False)
                mi = 1
                for t, d in TAPS:
                    if d == 0:
                        continue
                    co = 2 * t
                    nc.tensor.matmul(ofull, shifts[:, mi, :],
                                     A_sb[:, 0:NG, co:co + 2],
                                     start=False, stop=False)
                    last = (mi + 1 == 16)
                    if d > 0:
                        nc.tensor.matmul(ofull, shifts[:, mi + 1, :],
                                         A_sb[:, 1:NG + 1, co:co + 2],
                                         start=False, stop=last)
                    else:
                        nc.tensor.matmul(_rap(ps_off[:, 2], [(2, NG - 1), (1, 2)]),
                                         shifts[:, mi + 1, :],
                                         A_sb[:, 0:NG - 1, co:co + 2],
                                         start=False, stop=last)
                    mi += 2
                acc = ap_.tile([128, NG, 2], f32, tag="acc")
                nc.vector.tensor_copy(acc[:], ofull)

                # ---------------- index math ----------------
                pyx = ap_.tile([128, NG, 2], f32, tag="pyx")
                nc.vector.tensor_tensor(pyx[:], acc[:], base[:], Alu.add)
                nc.vector.tensor_scalar(pyx[:], pyx[:], 0.0, float(H - 1),
                                        Alu.max, Alu.min)
                y0i = ap_.tile([128, NG, 2], dt.int32, tag="y0i")
                nc.vector.tensor_copy(y0i[:], pyx[:])
                icast = ap_.tile([128, NG, 2], f32, tag="icast")
                nc.vector.tensor_copy(icast[:], y0i[:])
                gtt = ap_.tile([128, NG, 2], f32, tag="gtt")
                nc.vector.tensor_tensor(gtt[:], icast[:], pyx[:], Alu.is_gt)
                ifl = ap_.tile([128, NG, 2], f32, tag="ifl")
                nc.vector.tensor_tensor(ifl[:], icast[:], gtt[:], Alu.subtract)
                frac = ap_.tile([128, NG, 2], f32, tag="frac")
                nc.vector.tensor_tensor(frac[:], pyx[:], ifl[:], Alu.subtract)
                # token base id: Bt = 59 + 58*ifl_y + ifl_x
                Bt = ap_.tile([128, NG], f32, tag="Bt")
                nc.vector.tensor_scalar(Bt[:], ifl[:, :, 0], float(PW),
                                        float(PW + 1), Alu.mult, Alu.add)
                nc.vector.tensor_tensor(Bt[:], Bt[:], ifl[:, :, 1], Alu.add)
                # 16-wrap fold via PE one-hot matmuls (fp32, exact ints):
                # ps_idx[16j+q, 8g+m] = Bt[16m+q, g] for all j
                ps_idx = pst.tile([128, 224], f32, tag="pidx")
                for m in range(8):
                    nc.tensor.matmul(_rap(ps_idx[:, m], [(8, NG)]),
                                     pfold[:, m, :], Bt[:],
                                     start=True, stop=True)
                idxs = ip.tile([128, 2, 224], i16, tag="idxs")
                nc.vector.memset(idxs[:, :, 216:224], 0)
                nc.vector.tensor_scalar(idxs[:, 0, 0:216], ps_idx[:, 0:216],
                                        0.0, None, Alu.add)
                nc.vector.tensor_scalar(idxs[:, 1, 0:216], ps_idx[:, 0:216],
                                        float(PW), None, Alu.add)

                # ---------------- wx/wy broadcast ----------------
                w_bf = ip.tile([128, 2, NG], bf16, tag="wbf")   # [p, o, g]
                nc.vector.tensor_copy(
                    w_bf[:], _rap(frac[:, 0, 0], [(1, 2), (2, NG)]))
                ps_t = pst.tile([54, 128], bf16, tag="pst")
                nc.tensor.transpose(ps_t[:], _rap(w_bf[:, 0, 0], [(1, 54)]),
                                    ident[:])
                w_cols = wcp.tile([54, 128], bf16, tag="wcols")
                nc.scalar.copy(w_cols[:], ps_t[:])
                # HBM bounce: [54,128] col-major -> [2, NPOS] row layout
                w_hbm = dp.tile([54, 128], bf16, tag="whbm")
                nc.sync.dma_start(w_hbm[:], w_cols[:])
                w_rows = wcp.tile([2, NPOS2], bf16, tag="wrows")
                nc.vector.memset(w_rows[:, NPOS:NPOS2], 0.0)
                nc.sync.dma_start(
                    w_rows[:, 0:NPOS],
                    w_hbm[:].rearrange("(o g) p -> o (g p)", o=2))
                WY = wp.tile([128, NPOS2], bf16, tag="WY")
                WX = wp.tile([128, NPOS2], bf16, tag="WX")
                for o, Wt in ((0, WY), (1, WX)):
                    for n0 in range(0, NPOS2, 512):
                        n1 = min(n0 + 512, NPOS2)
                        pwt = ps.tile([128, 512], f32, tag="ps")
                        nc.tensor.matmul(pwt[:, 0:n1 - n0], sel[:, o, :],
                                         w_rows[:, n0:n1],
                                         start=True, stop=True)
                        if (n0 // 512) % 2 == 0:
                            nc.scalar.copy(Wt[:, n0:n1], pwt[:, 0:n1 - n0])
                        else:
                            nc.vector.tensor_copy(Wt[:, n0:n1], pwt[:, 0:n1 - n0])

                # ---------------- gather + combine per chunk ----------------
                g_sb = gsp.tile([128, 2, NPOS2], bf16, tag="gsb")
                NI16 = NIDX // 16
                tok_flat = _rap(tok[:, 0, 0], [(1, NG * 512)])
                for c2 in range(NCH):
                    Gt = gp.tile([128, 8, NIDX], bf16, tag="G")
                    for v in range(2):
                        nc.gpsimd.dma_gather(
                            Gt[:, 4 * v:4 * (v + 1), :],
                            tok_flat,
                            idxs[:, v, c2 * NI16:(c2 + 1) * NI16],
                            num_idxs=NIDX, num_idxs_reg=NIDX,
                            elem_size=512, transpose=True,
                            queue_num=(c2 * 2 + v) % NQ,
                            sbuf_tokens_per_rank=128,
                            sbuf_free_dim_per_rank=TOKB)
                    # x-lerp per v: L=[4v+0:4v+2], R=[4v+2:4v+4]
                    n0 = c2 * NIDX
                    WXc = _rap(WX[:, n0], [(0, 2), (1, NIDX)])
                    WYc = _rap(WY[:, n0], [(0, 2), (1, NIDX)])
                    tt = tp_.tile([128, 2, 2, NIDX], bf16, tag="tt")
                    for v in range(2):
                        L = Gt[:, 4 * v:4 * v + 2, :]
                        R = Gt[:, 4 * v + 2:4 * v + 4, :]
                        dd = sp.tile([128, 2, NIDX], bf16, tag="dd")
                        nc.vector.tensor_tensor(dd[:], R, L, Alu.subtract)
                        mm = sp.tile([128, 2, NIDX], bf16, tag="mm")
                        nc.vector.tensor_tensor(mm[:], dd[:], WXc, Alu.mult)
                        nc.vector.tensor_tensor(tt[:, v, :, :], L, mm[:],
                                                Alu.add)
                    # y-lerp
                    dy_ = sp.tile([128, 2, NIDX], bf16, tag="dd")
                    nc.vector.tensor_tensor(dy_[:], tt[:, 1, :, :],
                                            tt[:, 0, :, :], Alu.subtract)
                    my_ = sp.tile([128, 2, NIDX], bf16, tag="mm")
                    nc.vector.tensor_tensor(my_[:], dy_[:], WYc, Alu.mult)
                    nc.vector.tensor_tensor(
                        _rap(g_sb[:, 0, n0], [(NPOS2, 2), (1, NIDX)]),
                        tt[:, 0, :, :], my_[:], Alu.add)

                # ---------------- dw2 (2x2 stride-2) -------------------------
                d2_lo = d2p.tile([128, NOUT], bf16, tag="d2lo")
                d2_hi = d2p.tile([64, NOUT], bf16, tag="d2hi")
                glo = g_sb[:, 0, :]
                ghi = g_sb[0:64, 1, :]
                for obase, osz, dtile in ((0, 128, d2_lo), (128, 64, d2_hi)):
                    for ut in range(2):
                        pd = ps.tile([osz, UT], f32, tag="ps")
                        for t in range(4):
                            dy, dx = t // 2, t % 2
                            boff = PW * (1 + dy) + (1 + dx) + ut * 14 * 2 * PW
                            rhs_lo = _sv(glo, boff, 14, 2 * PW, 28, 2)
                            rhs_hi = _sv(ghi, boff, 14, 2 * PW, 28, 2)
                            nc.tensor.matmul(
                                pd[:], k2t_lo[:, t, obase:obase + osz], rhs_lo,
                                start=(t == 0), stop=False)
                            nc.tensor.matmul(
                                pd[:], k2t_hi[:, t, obase:obase + osz], rhs_hi,
                                start=False, stop=(t == 3))
                        nc.scalar.copy(dtile[:, ut * UT:(ut + 1) * UT], pd[:])

                # ---------------- pw2 ----------------------------------------
                out_sb = op.tile([128, 3, NOUT], bf16, tag="osb")
                for o3 in range(3):
                    osl = slice(o3 * 128, (o3 + 1) * 128)
                    for ut in range(2):
                        usl = slice(ut * UT, (ut + 1) * UT)
                        po = ps.tile([128, UT], f32, tag="ps")
                        nc.tensor.matmul(po[:], pw2t_lo[:, osl],
                                         d2_lo[:, usl], start=True, stop=False)
                        nc.tensor.matmul(po[:], pw2t_hi[:, osl],
                                         d2_hi[:, usl], start=False, stop=True)
                        nc.scalar.add(out_sb[:, o3, usl], po[:],
                                      bout[:, o3:o3 + 1])
                nc.sync.dma_start(
                    out_d[si, :, :].rearrange("(b p) n -> p b n", p=128),
                    out_sb[:])

    nc.compile()
    return nc


def _prep_inputs(p):
    x = p['x'].astype(np.float32)
    W1, b1, b_out = _fold_params(p)

    xpad = np.zeros((B, C, PH, PW), np.float32)
    xpad[:, :, 1:PH - 1, 1:PW - 1] = x
    xflat = np.zeros((B, C + 1, XW), np.float32)
    xflat[:, 0:C, 0:NPAD] = xpad.reshape(B, C, NPAD)
    xflat[:, C, :] = 1.0
    xflat = xflat.astype(BF16)

    poff = p['poff_w'].astype(np.float32)          # [2, C, 3, 3]
    wst = np.zeros((C, 18), np.float32)            # col = t*2 + o
    for t in range(9):
        dy, dx = t // 3, t % 3
        for o in range(2):
            wst[:, t * 2 + o] = poff[o, :, dy, dx]
    w1st = np.concatenate([W1.T, wst], axis=1)     # [C, 210]
    w1st_lo = np.ascontiguousarray(w1st[0:128]).astype(BF16)
    w1st_hi = np.zeros((65, C + 18), np.float32)
    w1st_hi[0:64] = w1st[128:192]
    w1st_hi[64, 0:C] = b1
    w1st_hi = w1st_hi.astype(BF16)

    dw2 = p['dw2_w'].astype(np.float32)            # [O, C, 2, 2]
    k2t = np.zeros((4, C, C), np.float32)
    for t in range(4):
        dy, dx = t // 2, t % 2
        k2t[t] = dw2[:, :, dy, dx].T               # [c, o]
    k2t = k2t.astype(BF16)

    pw2t = np.ascontiguousarray(p['pw2_w'].astype(np.float32).T).astype(BF16)
    bout = b_out.reshape(3, 128).astype(np.float32)

    s = np.arange(NPOS, dtype=np.float32)
    ypad = np.floor_divide(np.minimum(s, NPAD - 1), PW)
    xpad_c = np.minimum(s, NPAD - 1) % PW
    base = np.zeros((128, NG, 2), np.float32)
    base[:, :, 0] = (ypad - 1.0 + float(p['poff_b'][0])).reshape(NG, 128).T
    base[:, :, 1] = (xpad_c - 1.0 + float(p['poff_b'][1])).reshape(NG, 128).T

    sel = np.zeros((2, 2, 128), np.float32)
    sel[0, 0, :] = 1.0
    sel[1, 1, :] = 1.0
    sel = sel.astype(BF16)
    ident = np.eye(128, dtype=np.float32).astype(BF16)

    # one-hot shift matrices for the 9-tap sum: out[m] += A[m + d] via
    # lhsT[k, m] = 1 at k = m+d (main) / k = m+d-+128 (block-crossing wrap)
    shifts = np.zeros((17, 128, 128), np.float32)
    shifts[0] = np.eye(128)
    mi = 1
    for t in range(9):
        d = (t // 3 - 1) * PW + (t % 3 - 1)
        if d == 0:
            continue
        for m in range(128):
            k = m + d
            if 0 <= k < 128:
                shifts[mi, k, m] = 1.0
            kw = m + d - 128 if d > 0 else m + d + 128
            if 0 <= kw < 128:
                shifts[mi + 1, kw, m] = 1.0
        mi += 2
    shifts = shifts.astype(BF16)

    # fold matrices: ps_idx[16j+q, 8g+m] = Bt[16m+q, g]
    pfold = np.zeros((8, 128, 128), np.float32)
    for m in range(8):
        for mp in range(128):
            pfold[m, 16 * m + (mp % 16), mp] = 1.0

    shared = dict(w1st_lo=w1st_lo, w1st_hi=w1st_hi,
                  k2t=k2t, pw2t=pw2t, bout=bout, base=base, sel=sel,
                  ident=ident, shifts=shifts, pfold=pfold)
    in_maps = []
    for ci in range(NCORES):
        m = dict(shared)
        m['x'] = np.ascontiguousarray(xflat[ci * NS:(ci + 1) * NS])
        in_maps.append(m)
    return in_maps


def kernel(**inputs):
    from concourse.bass_utils import run_bass_kernel_spmd

    p = {k: np.asarray(v) for k, v in inputs.items()}
    in_maps = _prep_inputs(p)
    nc = build_nc()
    res = run_bass_kernel_spmd(nc, in_maps, core_ids=list(range(NCORES)))
    outs = [res.results[ci]['out'] for ci in range(NCORES)]
    out = np.concatenate([np.asarray(o).astype(np.float32) for o in outs],
                         axis=0)
    return out.reshape(B, CO, HOUT, WOUT)
